# revision 24
# baseline (speedup 1.0000x reference)
"""Trainium2 Bass kernel for nn_Encoder_83992380441041 (causal linear attention
encoder, last-position readout).

Math (per segment b of T tokens):
    yn   = LayerNorm(x_b) * gamma + beta          (beta == 0 here)
    K    = phi(yn @ Wk.T); V = yn @ Wv.T; q = phi(yn[T-1] @ Wq.T)
    out  = q @ (K.T V) / (q . sum_t K_t + eps)    [only last position matters]
with phi(a) = elu(a)+1 = min(exp(a),1) + relu(a).

v2 design (bf16 matmul pipeline):
  * gamma + LN centering folded into the weights (host):
    x @ (W - 1 s~/d) = (x - mu 1) @ W.
  * PE transposes raw x (f32r, 1.5 cyc/row); PSUM evac converts to bf16.
  * G = xT.T @ Wkv' in bf16 (1 cyc/row); r (=rsqrt(var+eps)) applied at
    PSUM evac with a broadcast tensor_tensor (r is per-token = per-partition
    in token-major G).
  * phi split: e=exp (ACT), min(e,1) + relu(z) on GPSIMD (idle otherwise).
  * bn_stats batched 4 tiles/op (FMAX=512); manual even/odd merge replaces
    per-tile bn_aggr.
  * q-path runs early (during the x DMA window) and doubles as the ACT
    table warmer (Sqrt + Exp).

Sharding: data-parallel over segments. 64 segments -> 8 cores x 8 segments.
"""

import numpy as np
import ml_dtypes

import concourse.bass as bass
import concourse.tile as tile
from concourse import mybir
from concourse.bass_utils import run_bass_kernel_spmd
from concourse.vector_clock import ScopedClock
import bass_rust

EPS_LN = 1e-5
EPS_DEN = 1e-5

F32 = mybir.dt.float32
F32R = mybir.dt.float32r
BF16 = mybir.dt.bfloat16
AF = mybir.ActivationFunctionType
ALU = mybir.AluOpType

N_CORES = 8


def _patched_drain_and_barrier(self, tick_clock, wait_clock):
    # Stock TileContext exit puts one sem-wait per outstanding proc on a
    # single InstDrain; walrus in this container caps sync waits per
    # instruction. Split them across a chain of drains on the same engine
    # (program order preserved => equivalent).
    nc = self.nc
    drain_inst = nc.sync.drain()
    wait_clock.add_sem_waits(
        drain_inst.ins, ScopedClock({None: tick_clock.global_clock})
    )
    si = drain_inst.ins.sync_info
    if si is not None and si.on_wait is not None and len(si.on_wait) > 1:
        waits = list(si.on_wait)
        si.on_wait = waits[:1]
        for w in waits[1:]:
            d2 = nc.sync.drain()
            si2 = d2.ins.sync_info
            if si2 is None:
                d2.ins.sync_info = bass_rust.SyncInfo(on_wait=[w], on_update=[])
            else:
                si2.on_wait = [w]
    nc.all_engine_barrier()
    assert self.sems is not None
    popped = nc._tile_sem_poison_stack.pop()
    assert popped is self._sem_poison
    nc.clear_and_free_semaphores(list(self.sems.allocated().values()))


tile.TileContext._drain_and_barrier = _patched_drain_and_barrier

_orig_commit = tile.TileContext._commit_instruction
_wsplit_counter = [0]


def _patched_commit_instruction(self, inst, lazy_reg_writes: bool = True):
    # Enforce the per-instruction sync-wait capacity of the walrus in this
    # container (1 for regular instructions, 2 for EventSemaphore) by
    # spilling excess waits onto same-engine NOPs committed just before.
    si = getattr(inst, "sync_info", None)
    if si is not None and si.on_wait:
        cap = 2 if isinstance(inst, mybir.InstEventSemaphore) else 1
        if len(si.on_wait) > cap:
            waits = list(si.on_wait)
            si.on_wait = waits[:cap]
            for w in waits[cap:]:
                _wsplit_counter[0] += 1
                nop = mybir.InstNoOp(
                    name=f"wsplit-{_wsplit_counter[0]}",
                    sync_info=mybir.SyncInfo(on_wait=[w], on_update=[]),
                    bass_nofuse=True,
                    engine=inst.engine,
                )
                _orig_commit(self, nop, lazy_reg_writes=False)
    return _orig_commit(self, inst, lazy_reg_writes=lazy_reg_writes)


tile.TileContext._commit_instruction = _patched_commit_instruction


def _build(n_tok: int, n_seg: int, d: int, f: int):
    """Per-core program. Inputs: x [n_tok,d] fp32; wpack bf16 [128, 576] =
    [wkv_bf(128) | wq_bf(64) | ident_bf(128) | ident_f32-as-2xbf16(256)].
    Output: z [n_seg, f] fp32."""
    P = 128
    assert n_tok % P == 0 and d == P
    n_tiles = n_tok // P
    t_seg = n_tok // n_seg
    assert t_seg % P == 0
    tiles_per_seg = t_seg // P
    f2 = 2 * f
    B = 4                       # tiles per block
    n_blk = n_tiles // B
    assert n_tiles % B == 0
    assert B == 2 * tiles_per_seg
    c = f2 + 1                  # kv columns per tile: [K | V | ones]

    WCOLS = f2 + f + P  # wkv + wq + ident_bf

    nc = bass.Bass()
    x_d = nc.declare_dram_parameter("x", [n_tok, d], F32R, isOutput=False)
    wpack_d = nc.declare_dram_parameter("wpack", [P, WCOLS], BF16, isOutput=False)
    identr_d = nc.declare_dram_parameter(
        "identr", [P, P], F32R, isOutput=False)
    z_d = nc.declare_dram_parameter("z", [f + 1, n_seg], F32, isOutput=True)

    with tile.TileContext(nc) as tc:
        with (
            tc.tile_pool(name="singles", bufs=1) as singles,
            tc.tile_pool(name="phi", bufs=2) as phip,
            tc.tile_pool(name="sseg", bufs=4) as ssegp,
            tc.tile_pool(name="fin", bufs=1) as finp,
            tc.tile_pool(name="psT", bufs=2, space="PSUM") as psT,
            tc.tile_pool(name="psG", bufs=2, space="PSUM") as psG,
            tc.tile_pool(name="psS", bufs=2, space="PSUM") as psS,
            tc.tile_pool(name="psM", bufs=1, space="PSUM") as psM,
        ):
            # --- persistent buffers ---
            xbig = singles.tile([P, n_tok], F32R)
            identr = singles.tile([P, P], F32R)
            wpack = singles.tile([P, WCOLS], BF16)
            xct_bf = singles.tile([P, n_tok], BF16)
            kvbig = singles.tile([P, n_tiles * c], BF16)
            bnbig = singles.tile([P, n_tiles, 6], F32)
            rbig = singles.tile([P, n_tiles], F32)
            mgtmp = singles.tile([P, 3 * n_tiles], F32)   # dm | s2 | v128
            eps_s = singles.tile([P, 1], F32)

            wkvb = wpack[:, 0:f2]
            wqb = wpack[:, f2:f2 + f]
            identb = wpack[:, f2 + f:f2 + f + P]
            idr = identr[:, 0:P]

            # --- DMA triggers: spread across engines for early starts ---
            xsrc = x_d.rearrange("(n p) d -> p n d", p=P)
            xlast = finp.tile([n_seg, d], F32)
            nc.sync.dma_start(
                out=xlast[:], in_=x_d[t_seg - 1::t_seg, :].bitcast(F32))
            nc.scalar.dma_start(out=identr[:], in_=identr_d[:])
            H = n_tiles // 2
            nc.sync.dma_start(
                out=xbig[:, 0:H * P], in_=xsrc[:, 0:H, :])
            nc.sync.dma_start(
                out=xbig[:, H * P:n_tok], in_=xsrc[:, H:n_tiles, :])
            nc.gpsimd.dma_start(out=wpack[:], in_=wpack_d[:])

            nc.vector.memset(eps_s[:], EPS_LN)
            nc.vector.memset(kvbig[:, f2::c], 1.0)

            # PE warm-up: HAM clock gate needs sustained matmul activity.
            junk = singles.tile([P, P], BF16)
            nc.vector.memset(junk[:], 0.0)
            for _ in range(12):
                wps = psM.tile([P, P], F32, tag="m")
                nc.tensor.matmul(
                    wps[:], lhsT=junk[:], rhs=junk[:],
                    start=True, stop=True, skip_group_check=True,
                )

            xview = xbig[:].bitcast(F32).rearrange("p (n d) -> p n d", d=P)
            xview_r = xbig[:].rearrange("p (n d) -> p n d", d=P)

            # ---------------- q-path part 1 (early; warms both ACT tables) -
            bn8 = finp.tile([n_seg, 6], F32)
            mg8 = finp.tile([n_seg, 3], F32)
            r8 = finp.tile([n_seg, 1], F32)
            xq_bf = finp.tile([n_seg, d], BF16)
            xqT = finp.tile([P, n_seg], BF16)
            eq = finp.tile([P, n_seg], BF16)
            em1q = finp.tile([P, n_seg], BF16)
            q2big = finp.tile([P, n_seg], BF16)
            qstack = finp.tile([P, n_seg], BF16)

            nc.vector.bn_stats(out=bn8[:], in_=xlast[:])
            nc.vector.tensor_tensor(
                out=mg8[:, 0:1], in0=bn8[:, 1:2], in1=bn8[:, 4:5],
                op=ALU.subtract)
            nc.vector.tensor_tensor(
                out=mg8[:, 1:2], in0=bn8[:, 2:3], in1=bn8[:, 5:6],
                op=ALU.add)
            nc.vector.tensor_tensor(
                out=mg8[:, 0:1], in0=mg8[:, 0:1], in1=mg8[:, 0:1],
                op=ALU.mult)
            nc.vector.scalar_tensor_tensor(
                out=mg8[:, 2:3], in0=mg8[:, 0:1], scalar=float(d) / 4.0,
                in1=mg8[:, 1:2], op0=ALU.mult, op1=ALU.add)
            # r8 = 1/sqrt(v128/d + eps)
            nc.scalar.activation(
                out=r8[:], in_=mg8[:, 2:3], func=AF.Sqrt,
                bias=eps_s[0:n_seg, :], scale=1.0 / float(d))
            nc.vector.reciprocal(out=r8[:], in_=r8[:])
            nc.scalar.activation(
                out=xq_bf[:], in_=xlast[:], func=AF.Copy, scale=r8[:])
            nc.vector.memset(qstack[:], 0.0)

            def q_path_part2():
                xqT_ps = psM.tile([P, n_seg], BF16, tag="m")
                nc.tensor.matmul(
                    xqT_ps[:], lhsT=xq_bf[:], rhs=identb[0:n_seg, 0:n_seg],
                    is_transpose=True, start=True, stop=True,
                    skip_group_check=True)
                nc.vector.tensor_copy(out=xqT[:], in_=xqT_ps[:])
                qc_ps = psM.tile([P, n_seg], F32, tag="m")
                nc.tensor.matmul(
                    qc_ps[0:f, :], lhsT=wqb, rhs=xqT[:],
                    start=True, stop=True, skip_group_check=True)
                nc.tensor.matmul(
                    qc_ps[f:2 * f, :], lhsT=wqb, rhs=xqT[:],
                    start=True, stop=True, skip_group_check=True,
                    tile_position=(0, f))
                nc.scalar.activation(out=eq[:], in_=qc_ps[:], func=AF.Exp)
                nc.vector.tensor_scalar_min(
                    out=em1q[:], in0=eq[:], scalar1=1.0)
                nc.vector.scalar_tensor_tensor(
                    out=q2big[:], in0=qc_ps[:], scalar=0.0, in1=em1q[:],
                    op0=ALU.max, op1=ALU.add)
                nc.vector.tensor_copy(
                    out=qstack[0:f, 0:n_seg:2], in_=q2big[0:f, 0:n_seg:2])
                nc.vector.tensor_copy(
                    out=qstack[f:2 * f, 1:n_seg:2],
                    in_=q2big[f:2 * f, 1:n_seg:2])

            # ---------------- stats for all tiles, then one merge ----------
            for n in range(n_tiles):
                nc.vector.bn_stats(
                    out=bnbig[:, n, :], in_=xview[:, n, :])
            dm = mgtmp[:, 0:n_tiles]
            s2 = mgtmp[:, n_tiles:2 * n_tiles]
            v128 = mgtmp[:, 2 * n_tiles:3 * n_tiles]
            nc.vector.tensor_tensor(
                out=dm.unsqueeze(-1), in0=bnbig[:, :, 1:2],
                in1=bnbig[:, :, 4:5], op=ALU.subtract)
            nc.vector.tensor_tensor(
                out=s2.unsqueeze(-1), in0=bnbig[:, :, 2:3],
                in1=bnbig[:, :, 5:6], op=ALU.add)
            nc.vector.tensor_tensor(out=dm, in0=dm, in1=dm, op=ALU.mult)
            nc.vector.scalar_tensor_tensor(
                out=v128, in0=dm, scalar=float(d) / 4.0, in1=s2,
                op0=ALU.mult, op1=ALU.add)

            # ---------------- phase A: transpose + evac all blocks ---------
            for b in range(n_blk):
                b0 = b * B
                pT = psT.tile([P, B * P], F32R)
                for j in range(B):
                    n = b0 + j
                    nc.tensor.matmul(
                        pT[:, j * P:(j + 1) * P],
                        lhsT=xview_r[:, n, :],
                        rhs=idr, is_transpose=True,
                        start=True, stop=True, skip_group_check=True)
                dst = xct_bf[:, b0 * P:(b0 + B) * P]
                nc.scalar.copy(out=dst, in_=pT[:].bitcast(F32))

            # r = 1/sqrt(v128/d + eps)
            nc.scalar.activation(
                out=rbig[:], in_=v128, func=AF.Sqrt,
                bias=eps_s[:], scale=1.0 / float(d))
            nc.vector.reciprocal(out=rbig[:], in_=rbig[:])

            # ---------------- phase B: G / scale / phi / S per block -------
            ndT = psM.tile([f + 1, n_seg], F32, tag="nd")
            s_sbs = []
            for b in range(n_blk):
                b0 = b * B
                # G = x @ Wkv' (centered via weight fold), token-major
                gT = psG.tile([P, B * f2], F32)
                for j in range(B):
                    n = b0 + j
                    nc.tensor.matmul(
                        gT[:, j * f2:(j + 1) * f2],
                        lhsT=xct_bf[:, n * P:(n + 1) * P], rhs=wkvb,
                        start=True, stop=True, skip_group_check=True)

                # kv = r * G for the whole block in one broadcast mult
                kv_blk = kvbig[:, b0 * c:(b0 + B) * c].rearrange(
                    "p (j e) -> p j e", e=c)
                rview = rbig[:, b0:b0 + B].unsqueeze(-1).broadcast_to(
                    [P, B, f2])
                nc.vector.tensor_tensor(
                    out=kv_blk[:, :, 0:f2],
                    in0=gT[:].rearrange("p (j e) -> p j e", e=f2),
                    in1=rview, op=ALU.mult)

                # phi on the K halves: e=exp(z) [ACT]; em1=min(e,1) [DVE];
                # kh = relu(z) + em1 [DVE, in place on the kv K columns]
                kh = kv_blk[:, :, 0:f]
                e_t = phip.tile([P, B * f], BF16, tag="e")
                nc.scalar.activation(out=e_t[:], in_=kh, func=AF.Exp)
                em1 = phip.tile([P, B * f], BF16, tag="m")
                nc.vector.tensor_scalar_min(out=em1[:], in0=e_t[:], scalar1=1.0)
                nc.vector.scalar_tensor_tensor(
                    out=kh, in0=kh, scalar=0.0,
                    in1=em1[:].rearrange("p (j e) -> p j e", e=f),
                    op0=ALU.max, op1=ALU.add)

                # S|Z for the block's two segments, column-packed in the PE
                s_ps = psS.tile([P, f + 1], F32)
                for hh in range(2):
                    s = 2 * b + hh
                    for j in range(tiles_per_seg):
                        n = s * tiles_per_seg + j
                        nc.tensor.matmul(
                            s_ps[hh * f:(hh + 1) * f, :],
                            lhsT=kvbig[:, n * c:n * c + f],
                            rhs=kvbig[:, n * c + f:(n + 1) * c],
                            start=(j == 0), stop=(j == tiles_per_seg - 1),
                            skip_group_check=True,
                            tile_position=(0, hh * f))
                s_sb = ssegp.tile([P, f + 1], BF16)
                nc.scalar.copy(out=s_sb[:], in_=s_ps[:])
                s_sbs.append(s_sb)

            # q phi after all main-phi Exps (keeps ACT on one table run)
            q_path_part2()
            for b in range(n_blk):
                nc.tensor.matmul(
                    ndT[:, 2 * b:2 * b + 2], lhsT=s_sbs[b][:],
                    rhs=qstack[:, 2 * b:2 * b + 2],
                    start=True, stop=True, skip_group_check=True)

            # ---------------- readout: ship raw [num|den]^T; divide on host
            ndsb = finp.tile([f + 1, n_seg], F32)
            nc.scalar.copy(out=ndsb[:], in_=ndT[:])
            nc.sync.dma_start(out=z_d[:], in_=ndsb[:])

    return nc


def _to_bf16_u16(a64):
    return np.asarray(a64, dtype=ml_dtypes.bfloat16).view(np.uint16)


def _prep(inputs):
    x = np.ascontiguousarray(np.asarray(inputs["x"], dtype=np.float32))
    batch = np.asarray(inputs["batch"]).astype(np.int64)
    gamma = np.asarray(inputs["gamma"], dtype=np.float32)
    beta = np.asarray(inputs["beta"], dtype=np.float32)
    wk = np.asarray(inputs["Wk"], dtype=np.float32)
    wq = np.asarray(inputs["Wq"], dtype=np.float32)
    wv = np.asarray(inputs["Wv"], dtype=np.float32)
    n_batches = int(np.asarray(inputs["n_batches"]))

    n, d = x.shape
    f = wk.shape[0]
    t_seg = n // n_batches
    counts = np.bincount(batch, minlength=n_batches)
    if not (np.all(counts == t_seg) and np.all(np.diff(batch) >= 0)):
        raise NotImplementedError("kernel specialized for equal sorted segments")
    if np.any(beta != 0.0):
        raise NotImplementedError("kernel specialized for beta == 0")

    wkg = (wk * gamma[None, :]).astype(np.float64)
    wvg = (wv * gamma[None, :]).astype(np.float64)
    wqg = (wq * gamma[None, :]).astype(np.float64)
    wkv_t = np.concatenate([wkg, wvg], axis=0).T            # [d, 2f]
    wq_t = wqg.T                                            # [d, f]
    # fold the LN centering into the weights:
    #   x @ (W - 1 s~/d) = (x - mu 1) @ W   since 1 @ W = colsums(W)
    wkv_t = wkv_t - wkv_t.sum(axis=0, keepdims=True) / d
    wq_t = wq_t - wq_t.sum(axis=0, keepdims=True) / d
    ident = np.eye(128, dtype=np.float32)

    # wpack (bf16 cols): [wkv(128) | wq(64) | ident_bf(128)]
    parts = [
        _to_bf16_u16(wkv_t),                 # [128, 128]
        _to_bf16_u16(wq_t),                  # [128, 64]
        _to_bf16_u16(ident),                 # [128, 128]
    ]
    wpack_u16 = np.ascontiguousarray(np.concatenate(parts, axis=1))
    wpack = wpack_u16.view(ml_dtypes.bfloat16)
    identr = np.ascontiguousarray(ident)

    return x, wpack, identr, n, d, f, n_batches, t_seg


def _run(inputs, trace=False):
    x, wpack, identr, n, d, f, n_batches, t_seg = _prep(inputs)

    segs_per_core = n_batches // N_CORES
    tok_per_core = segs_per_core * t_seg
    nc = _build(tok_per_core, segs_per_core, d, f)

    in_maps = []
    for cc in range(N_CORES):
        m = {
            "x": np.ascontiguousarray(
                x[cc * tok_per_core:(cc + 1) * tok_per_core]),
            "wpack": wpack,
            "identr": identr,
        }
        in_maps.append(m)

    res = run_bass_kernel_spmd(nc, in_maps, list(range(N_CORES)), trace=trace)
    zs = []
    for cc in range(N_CORES):
        nd = np.asarray(res.results[cc]["z"])        # [f+1, segs]
        zs.append(nd[:f, :].T / (nd[f, :][:, None] + EPS_DEN))
    z = np.concatenate(zs, axis=0).astype(np.float32)
    return z, res


def kernel(**inputs) -> np.ndarray:
    z, _ = _run(inputs, trace=False)
    return z


# revision 25
# speedup vs baseline: 1.0634x; 1.0634x over previous
"""Trainium2 Bass kernel for nn_Encoder_83992380441041 (causal linear attention
encoder, last-position readout).

Math (per segment b of T tokens):
    yn   = LayerNorm(x_b) * gamma + beta          (beta == 0 here)
    K    = phi(yn @ Wk.T); V = yn @ Wv.T; q = phi(yn[T-1] @ Wq.T)
    out  = q @ (K.T V) / (q . sum_t K_t + eps)    [only last position matters]
with phi(a) = elu(a)+1 = min(exp(a),1) + relu(a).

Design (bf16 matmul pipeline):
  * gamma + LN centering folded into the weights (host):
    x @ (W - 1 s~/d) = (x - mu 1) @ W, so the device never centers x.
  * PE transposes raw x in f32r (x and the identity are declared as f32r
    DRAM params to satisfy the walrus f32r-producer check); the PSUM evac
    (ACT copy) converts to bf16.
  * G = xT.T @ Wkv' in bf16 (1 cyc/row); r (=rsqrt(var+eps)) applied to
    the whole G block with ONE broadcast tensor_tensor (stride-0 AP; r is
    per-token = per-partition in token-major G).
  * phi = min(exp(z),1) + relu(z): Exp on ACT, min + relu-add on DVE.
    ACT table thrash is minimized by grouping Sqrt uses before Exp uses
    (each function switch costs a 1.28us table load).
  * per-tile bn_stats (walrus requires 6 els/partition out) + one batched
    manual even/odd merge (replaces 16 bn_aggr with 4 ops on [128,16]).
  * x loaded in 2 big DMAs on the sync queue (earlier first-half sem);
    weights via gpsimd SWDGE; only [num|den]^T is shipped out -- the
    final divide happens on host, trimming the device-side readout tail.

Sharding: data-parallel over segments. 64 segments -> 8 cores x 8 segments.
"""

import numpy as np
import ml_dtypes

import concourse.bass as bass
import concourse.tile as tile
from concourse import mybir
from concourse.bass_utils import run_bass_kernel_spmd
from concourse.vector_clock import ScopedClock
import bass_rust

EPS_LN = 1e-5
EPS_DEN = 1e-5

F32 = mybir.dt.float32
F32R = mybir.dt.float32r
BF16 = mybir.dt.bfloat16
AF = mybir.ActivationFunctionType
ALU = mybir.AluOpType

N_CORES = 8


def _patched_drain_and_barrier(self, tick_clock, wait_clock):
    # Stock TileContext exit puts one sem-wait per outstanding proc on a
    # single InstDrain; walrus in this container caps sync waits per
    # instruction. Split them across a chain of drains on the same engine
    # (program order preserved => equivalent).
    nc = self.nc
    drain_inst = nc.sync.drain()
    wait_clock.add_sem_waits(
        drain_inst.ins, ScopedClock({None: tick_clock.global_clock})
    )
    si = drain_inst.ins.sync_info
    if si is not None and si.on_wait is not None and len(si.on_wait) > 1:
        waits = list(si.on_wait)
        si.on_wait = waits[:1]
        for w in waits[1:]:
            d2 = nc.sync.drain()
            si2 = d2.ins.sync_info
            if si2 is None:
                d2.ins.sync_info = bass_rust.SyncInfo(on_wait=[w], on_update=[])
            else:
                si2.on_wait = [w]
    nc.all_engine_barrier()
    assert self.sems is not None
    popped = nc._tile_sem_poison_stack.pop()
    assert popped is self._sem_poison
    nc.clear_and_free_semaphores(list(self.sems.allocated().values()))


tile.TileContext._drain_and_barrier = _patched_drain_and_barrier

_orig_commit = tile.TileContext._commit_instruction
_wsplit_counter = [0]


def _patched_commit_instruction(self, inst, lazy_reg_writes: bool = True):
    # Enforce the per-instruction sync-wait capacity of the walrus in this
    # container (1 for regular instructions, 2 for EventSemaphore) by
    # spilling excess waits onto same-engine NOPs committed just before.
    si = getattr(inst, "sync_info", None)
    if si is not None and si.on_wait:
        cap = 2 if isinstance(inst, mybir.InstEventSemaphore) else 1
        if len(si.on_wait) > cap:
            waits = list(si.on_wait)
            si.on_wait = waits[:cap]
            for w in waits[cap:]:
                _wsplit_counter[0] += 1
                nop = mybir.InstNoOp(
                    name=f"wsplit-{_wsplit_counter[0]}",
                    sync_info=mybir.SyncInfo(on_wait=[w], on_update=[]),
                    bass_nofuse=True,
                    engine=inst.engine,
                )
                _orig_commit(self, nop, lazy_reg_writes=False)
    return _orig_commit(self, inst, lazy_reg_writes=lazy_reg_writes)


tile.TileContext._commit_instruction = _patched_commit_instruction


def _build(n_tok: int, n_seg: int, d: int, f: int):
    """Per-core program. Inputs: x [n_tok,d] fp32; wpack bf16 [128, 576] =
    [wkv_bf(128) | wq_bf(64) | ident_bf(128) | ident_f32-as-2xbf16(256)].
    Output: z [n_seg, f] fp32."""
    P = 128
    assert n_tok % P == 0 and d == P
    n_tiles = n_tok // P
    t_seg = n_tok // n_seg
    assert t_seg % P == 0
    tiles_per_seg = t_seg // P
    f2 = 2 * f
    B = 4                       # tiles per block
    n_blk = n_tiles // B
    assert n_tiles % B == 0
    assert B == 2 * tiles_per_seg
    c = f2 + 1                  # kv columns per tile: [K | V | ones]

    WCOLS = f2 + f + P  # wkv + wq + ident_bf

    nc = bass.Bass()
    x_d = nc.declare_dram_parameter("x", [n_tok, d], F32R, isOutput=False)
    wpack_d = nc.declare_dram_parameter("wpack", [P, WCOLS], BF16, isOutput=False)
    identr_d = nc.declare_dram_parameter(
        "identr", [P, P], F32R, isOutput=False)
    z_d = nc.declare_dram_parameter("z", [f + 1, n_seg], F32, isOutput=True)

    with tile.TileContext(nc) as tc:
        with (
            tc.tile_pool(name="singles", bufs=1) as singles,
            tc.tile_pool(name="phi", bufs=2) as phip,
            tc.tile_pool(name="sseg", bufs=4) as ssegp,
            tc.tile_pool(name="fin", bufs=1) as finp,
            tc.tile_pool(name="psT", bufs=2, space="PSUM") as psT,
            tc.tile_pool(name="psG", bufs=2, space="PSUM") as psG,
            tc.tile_pool(name="psS", bufs=2, space="PSUM") as psS,
            tc.tile_pool(name="psM", bufs=1, space="PSUM") as psM,
        ):
            # --- persistent buffers ---
            xbig = singles.tile([P, n_tok], F32R)
            identr = singles.tile([P, P], F32R)
            wpack = singles.tile([P, WCOLS], BF16)
            xct_bf = singles.tile([P, n_tok], BF16)
            kvbig = singles.tile([P, n_tiles * c], BF16)
            bnbig = singles.tile([P, n_tiles, 6], F32)
            rbig = singles.tile([P, n_tiles], F32)
            mgtmp = singles.tile([P, 3 * n_tiles], F32)   # dm | s2 | v128
            eps_s = singles.tile([P, 1], F32)

            wkvb = wpack[:, 0:f2]
            wqb = wpack[:, f2:f2 + f]
            identb = wpack[:, f2 + f:f2 + f + P]
            idr = identr[:, 0:P]

            # --- DMA triggers: spread across engines for early starts ---
            xsrc = x_d.rearrange("(n p) d -> p n d", p=P)
            xlast = finp.tile([n_seg, d], F32)
            nc.sync.dma_start(
                out=xlast[:], in_=x_d[t_seg - 1::t_seg, :].bitcast(F32))
            nc.scalar.dma_start(out=identr[:], in_=identr_d[:])
            H = n_tiles // 2
            nc.sync.dma_start(
                out=xbig[:, 0:H * P], in_=xsrc[:, 0:H, :])
            nc.sync.dma_start(
                out=xbig[:, H * P:n_tok], in_=xsrc[:, H:n_tiles, :])
            nc.gpsimd.dma_start(out=wpack[:], in_=wpack_d[:])

            nc.vector.memset(eps_s[:], EPS_LN)
            nc.vector.memset(kvbig[:, f2::c], 1.0)

            # PE warm-up: HAM clock gate needs sustained matmul activity.
            junk = singles.tile([P, P], BF16)
            nc.vector.memset(junk[:], 0.0)
            for _ in range(12):
                wps = psM.tile([P, P], F32, tag="m")
                nc.tensor.matmul(
                    wps[:], lhsT=junk[:], rhs=junk[:],
                    start=True, stop=True, skip_group_check=True,
                )

            xview = xbig[:].bitcast(F32).rearrange("p (n d) -> p n d", d=P)
            xview_r = xbig[:].rearrange("p (n d) -> p n d", d=P)

            # ---------------- q-path part 1 (early; warms both ACT tables) -
            bn8 = finp.tile([n_seg, 6], F32)
            mg8 = finp.tile([n_seg, 3], F32)
            r8 = finp.tile([n_seg, 1], F32)
            xq_bf = finp.tile([n_seg, d], BF16)
            xqT = finp.tile([P, n_seg], BF16)
            eq = finp.tile([P, n_seg], BF16)
            em1q = finp.tile([P, n_seg], BF16)
            q2big = finp.tile([P, n_seg], BF16)
            qstack = finp.tile([P, n_seg], BF16)

            nc.vector.bn_stats(out=bn8[:], in_=xlast[:])
            nc.vector.tensor_tensor(
                out=mg8[:, 0:1], in0=bn8[:, 1:2], in1=bn8[:, 4:5],
                op=ALU.subtract)
            nc.vector.tensor_tensor(
                out=mg8[:, 1:2], in0=bn8[:, 2:3], in1=bn8[:, 5:6],
                op=ALU.add)
            nc.vector.tensor_tensor(
                out=mg8[:, 0:1], in0=mg8[:, 0:1], in1=mg8[:, 0:1],
                op=ALU.mult)
            nc.vector.scalar_tensor_tensor(
                out=mg8[:, 2:3], in0=mg8[:, 0:1], scalar=float(d) / 4.0,
                in1=mg8[:, 1:2], op0=ALU.mult, op1=ALU.add)
            # r8 = 1/sqrt(v128/d + eps)
            nc.scalar.activation(
                out=r8[:], in_=mg8[:, 2:3], func=AF.Sqrt,
                bias=eps_s[0:n_seg, :], scale=1.0 / float(d))
            nc.vector.reciprocal(out=r8[:], in_=r8[:])
            nc.scalar.activation(
                out=xq_bf[:], in_=xlast[:], func=AF.Copy, scale=r8[:])
            nc.vector.memset(qstack[:], 0.0)

            def q_path_part2():
                xqT_ps = psM.tile([P, n_seg], BF16, tag="m")
                nc.tensor.matmul(
                    xqT_ps[:], lhsT=xq_bf[:], rhs=identb[0:n_seg, 0:n_seg],
                    is_transpose=True, start=True, stop=True,
                    skip_group_check=True)
                nc.vector.tensor_copy(out=xqT[:], in_=xqT_ps[:])
                qc_ps = psM.tile([P, n_seg], F32, tag="m")
                nc.tensor.matmul(
                    qc_ps[0:f, :], lhsT=wqb, rhs=xqT[:],
                    start=True, stop=True, skip_group_check=True)
                nc.tensor.matmul(
                    qc_ps[f:2 * f, :], lhsT=wqb, rhs=xqT[:],
                    start=True, stop=True, skip_group_check=True,
                    tile_position=(0, f))
                nc.scalar.activation(out=eq[:], in_=qc_ps[:], func=AF.Exp)
                nc.vector.tensor_scalar_min(
                    out=em1q[:], in0=eq[:], scalar1=1.0)
                nc.vector.scalar_tensor_tensor(
                    out=q2big[:], in0=qc_ps[:], scalar=0.0, in1=em1q[:],
                    op0=ALU.max, op1=ALU.add)
                nc.vector.tensor_copy(
                    out=qstack[0:f, 0:n_seg:2], in_=q2big[0:f, 0:n_seg:2])
                nc.vector.tensor_copy(
                    out=qstack[f:2 * f, 1:n_seg:2],
                    in_=q2big[f:2 * f, 1:n_seg:2])

            # ---------------- stats for all tiles, then one merge ----------
            for n in range(n_tiles):
                nc.vector.bn_stats(
                    out=bnbig[:, n, :], in_=xview[:, n, :])
            dm = mgtmp[:, 0:n_tiles]
            s2 = mgtmp[:, n_tiles:2 * n_tiles]
            v128 = mgtmp[:, 2 * n_tiles:3 * n_tiles]
            nc.vector.tensor_tensor(
                out=dm.unsqueeze(-1), in0=bnbig[:, :, 1:2],
                in1=bnbig[:, :, 4:5], op=ALU.subtract)
            nc.vector.tensor_tensor(
                out=s2.unsqueeze(-1), in0=bnbig[:, :, 2:3],
                in1=bnbig[:, :, 5:6], op=ALU.add)
            nc.vector.tensor_tensor(out=dm, in0=dm, in1=dm, op=ALU.mult)
            nc.vector.scalar_tensor_tensor(
                out=v128, in0=dm, scalar=float(d) / 4.0, in1=s2,
                op0=ALU.mult, op1=ALU.add)

            # ---------------- phase A: transpose + evac all blocks ---------
            for b in range(n_blk):
                b0 = b * B
                pT = psT.tile([P, B * P], F32R)
                for j in range(B):
                    n = b0 + j
                    nc.tensor.matmul(
                        pT[:, j * P:(j + 1) * P],
                        lhsT=xview_r[:, n, :],
                        rhs=idr, is_transpose=True,
                        start=True, stop=True, skip_group_check=True)
                dst = xct_bf[:, b0 * P:(b0 + B) * P]
                nc.scalar.copy(out=dst, in_=pT[:].bitcast(F32))

            # r = 1/sqrt(v128/d + eps)
            nc.scalar.activation(
                out=rbig[:], in_=v128, func=AF.Sqrt,
                bias=eps_s[:], scale=1.0 / float(d))
            nc.vector.reciprocal(out=rbig[:], in_=rbig[:])

            # ---------------- phase B: G / scale / phi / S per block -------
            ndT = psM.tile([f + 1, n_seg], F32, tag="nd")
            s_sbs = []
            for b in range(n_blk):
                b0 = b * B
                # G = x @ Wkv' (centered via weight fold), token-major
                gT = psG.tile([P, B * f2], F32)
                for j in range(B):
                    n = b0 + j
                    nc.tensor.matmul(
                        gT[:, j * f2:(j + 1) * f2],
                        lhsT=xct_bf[:, n * P:(n + 1) * P], rhs=wkvb,
                        start=True, stop=True, skip_group_check=True)

                # kv = r * G for the whole block in one broadcast mult
                kv_blk = kvbig[:, b0 * c:(b0 + B) * c].rearrange(
                    "p (j e) -> p j e", e=c)
                rview = rbig[:, b0:b0 + B].unsqueeze(-1).broadcast_to(
                    [P, B, f2])
                nc.vector.tensor_tensor(
                    out=kv_blk[:, :, 0:f2],
                    in0=gT[:].rearrange("p (j e) -> p j e", e=f2),
                    in1=rview, op=ALU.mult)

                # phi on the K halves: e=exp(z) [ACT]; em1=min(e,1) [DVE];
                # kh = relu(z) + em1 [DVE, in place on the kv K columns]
                kh = kv_blk[:, :, 0:f]
                e_t = phip.tile([P, B * f], BF16, tag="e")
                nc.scalar.activation(out=e_t[:], in_=kh, func=AF.Exp)
                em1 = phip.tile([P, B * f], BF16, tag="m")
                nc.vector.tensor_scalar_min(out=em1[:], in0=e_t[:], scalar1=1.0)
                nc.vector.scalar_tensor_tensor(
                    out=kh, in0=kh, scalar=0.0,
                    in1=em1[:].rearrange("p (j e) -> p j e", e=f),
                    op0=ALU.max, op1=ALU.add)

                # S|Z for the block's two segments, column-packed in the PE
                s_ps = psS.tile([P, f + 1], F32)
                for hh in range(2):
                    s = 2 * b + hh
                    for j in range(tiles_per_seg):
                        n = s * tiles_per_seg + j
                        nc.tensor.matmul(
                            s_ps[hh * f:(hh + 1) * f, :],
                            lhsT=kvbig[:, n * c:n * c + f],
                            rhs=kvbig[:, n * c + f:(n + 1) * c],
                            start=(j == 0), stop=(j == tiles_per_seg - 1),
                            skip_group_check=True,
                            tile_position=(0, hh * f))
                s_sb = ssegp.tile([P, f + 1], BF16)
                nc.scalar.copy(out=s_sb[:], in_=s_ps[:])
                s_sbs.append(s_sb)

            # q phi after all main-phi Exps (keeps ACT on one table run)
            q_path_part2()
            for b in range(n_blk):
                nc.tensor.matmul(
                    ndT[:, 2 * b:2 * b + 2], lhsT=s_sbs[b][:],
                    rhs=qstack[:, 2 * b:2 * b + 2],
                    start=True, stop=True, skip_group_check=True)

            # ---------------- readout: ship raw [num|den]^T; divide on host
            ndsb = finp.tile([f + 1, n_seg], F32)
            nc.scalar.copy(out=ndsb[:], in_=ndT[:])
            nc.sync.dma_start(out=z_d[:], in_=ndsb[:])

    return nc


def _to_bf16_u16(a64):
    return np.asarray(a64, dtype=ml_dtypes.bfloat16).view(np.uint16)


def _prep(inputs):
    x = np.ascontiguousarray(np.asarray(inputs["x"], dtype=np.float32))
    batch = np.asarray(inputs["batch"]).astype(np.int64)
    gamma = np.asarray(inputs["gamma"], dtype=np.float32)
    beta = np.asarray(inputs["beta"], dtype=np.float32)
    wk = np.asarray(inputs["Wk"], dtype=np.float32)
    wq = np.asarray(inputs["Wq"], dtype=np.float32)
    wv = np.asarray(inputs["Wv"], dtype=np.float32)
    n_batches = int(np.asarray(inputs["n_batches"]))

    n, d = x.shape
    f = wk.shape[0]
    t_seg = n // n_batches
    counts = np.bincount(batch, minlength=n_batches)
    if not (np.all(counts == t_seg) and np.all(np.diff(batch) >= 0)):
        raise NotImplementedError("kernel specialized for equal sorted segments")
    if np.any(beta != 0.0):
        raise NotImplementedError("kernel specialized for beta == 0")

    wkg = (wk * gamma[None, :]).astype(np.float64)
    wvg = (wv * gamma[None, :]).astype(np.float64)
    wqg = (wq * gamma[None, :]).astype(np.float64)
    wkv_t = np.concatenate([wkg, wvg], axis=0).T            # [d, 2f]
    wq_t = wqg.T                                            # [d, f]
    # fold the LN centering into the weights:
    #   x @ (W - 1 s~/d) = (x - mu 1) @ W   since 1 @ W = colsums(W)
    wkv_t = wkv_t - wkv_t.sum(axis=0, keepdims=True) / d
    wq_t = wq_t - wq_t.sum(axis=0, keepdims=True) / d
    ident = np.eye(128, dtype=np.float32)

    # wpack (bf16 cols): [wkv(128) | wq(64) | ident_bf(128)]
    parts = [
        _to_bf16_u16(wkv_t),                 # [128, 128]
        _to_bf16_u16(wq_t),                  # [128, 64]
        _to_bf16_u16(ident),                 # [128, 128]
    ]
    wpack_u16 = np.ascontiguousarray(np.concatenate(parts, axis=1))
    wpack = wpack_u16.view(ml_dtypes.bfloat16)
    identr = np.ascontiguousarray(ident)

    return x, wpack, identr, n, d, f, n_batches, t_seg


def _run(inputs, trace=False):
    x, wpack, identr, n, d, f, n_batches, t_seg = _prep(inputs)

    segs_per_core = n_batches // N_CORES
    tok_per_core = segs_per_core * t_seg
    nc = _build(tok_per_core, segs_per_core, d, f)

    in_maps = []
    for cc in range(N_CORES):
        m = {
            "x": np.ascontiguousarray(
                x[cc * tok_per_core:(cc + 1) * tok_per_core]),
            "wpack": wpack,
            "identr": identr,
        }
        in_maps.append(m)

    res = run_bass_kernel_spmd(nc, in_maps, list(range(N_CORES)), trace=trace)
    zs = []
    for cc in range(N_CORES):
        nd = np.asarray(res.results[cc]["z"])        # [f+1, segs]
        zs.append(nd[:f, :].T / (nd[f, :][:, None] + EPS_DEN))
    z = np.concatenate(zs, axis=0).astype(np.float32)
    return z, res


def kernel(**inputs) -> np.ndarray:
    z, _ = _run(inputs, trace=False)
    return z
